# revision 1
# baseline (speedup 1.0000x reference)
"""Trainium2 Bass kernel for nn_Attention_85074712199827.

Computes, for hidden [1,32,1024], encoder_outputs [32,2048,1024],
W_attn [1024,2048], b_attn [1024], v [1024]:

    h_proj  = hidden[0] @ W_attn[:, :1024].T
    e_proj  = encoder_outputs @ W_attn[:, 1024:].T
    energy  = tanh(e_proj + h_proj[:, None, :] + b_attn)
    att     = energy @ v
    out     = softmax(att, axis=1)          # [32, 2048] float32

Distribution: data-parallel over the batch across 8 NeuronCores (4
batch rows per core); the tiny parameters are replicated (pre-laid-out
and pre-cast to bf16 on the host). Each core runs an independent
Bass/Tile program; results are concatenated on the host.

Self-contained: only environment packages (concourse, numpy, ml_dtypes)
are imported; all shapes/sharding are hardcoded for this problem.
"""

from contextlib import ExitStack

import ml_dtypes
import numpy as np

import concourse.bass as bass
import concourse.tile as tile
from concourse import bacc, mybir

F32 = mybir.dt.float32
BF16 = mybir.dt.bfloat16
AF = mybir.ActivationFunctionType
P = 128


def build_nc(b_loc=4, s=2048, h=1024, n_cores=8, sb=512,
             transpose_mode="sbuf", warmup_mm=32,
             pe_bufs=5, encT_bufs=5, inp_bufs=3, bfp_bufs=5,
             group_units=4, first_units=3, keepalive_mm=0, startup_keep=12):
    n_sb = s // sb          # s-blocks per batch
    n_hc = h // P           # contraction chunks
    n_ot = h // P           # output (o) tiles
    si_n = sb // P          # 128-row subtiles per s-block
    n_sc = sb // 512        # 512-wide psum chunks per s-block
    SC = 512

    nc = bacc.Bacc("TRN2", target_bir_lowering=False, debug=False,
                   num_devices=n_cores)

    wt = nc.dram_tensor("wt", [2 * h, h], BF16, kind="ExternalInput").ap()
    hiddenT = nc.dram_tensor("hiddenT", [h, b_loc], BF16, kind="ExternalInput").ap()
    b_attn = nc.dram_tensor("b_attn", [h], F32, kind="ExternalInput").ap()
    v = nc.dram_tensor("v", [h], BF16, kind="ExternalInput").ap()
    enc = nc.dram_tensor("enc", [b_loc, s, h], F32, kind="ExternalInput").ap()
    out = nc.dram_tensor("out", [b_loc, s], F32, kind="ExternalOutput").ap()

    with tile.TileContext(nc) as tc, ExitStack() as ctx:
        const = ctx.enter_context(tc.tile_pool(name="const", bufs=1))
        psmall = ctx.enter_context(tc.tile_pool(name="psmall", bufs=1, space="PSUM"))

        # ---- PE warmup: dependency-free matmuls to lift the HAM clock
        # gate to 8/8 while the first enc block is still in flight ----
        if warmup_mm:
            wz = const.tile([P, SC], BF16)
            nc.gpsimd.memset(wz[:], 0)
            for i in range(warmup_mm):
                pw = psmall.tile([P, SC], F32, name="pw", tag="ps")
                nc.tensor.matmul(pw[:], wz[:, :P], wz[:], start=True, stop=True)

        # ---- small constants first (tiny; keep them off the critical
        # xbar-drain path) ----
        hT_bf = const.tile([P, n_hc, b_loc], BF16)
        nc.scalar.dma_start(hT_bf[:], hiddenT.rearrange("(hc p) b -> p hc b", p=P))

        baT = const.tile([P, n_ot], F32)
        nc.scalar.dma_start(baT[:], b_attn.rearrange("(oc p) -> p oc", p=P))

        vt_bf = const.tile([P, n_ot], BF16)
        nc.scalar.dma_start(vt_bf[:], v.rearrange("(oc p) -> p oc", p=P))

        # ---- weights: W_attn.T arrives [2h, h] bf16; Wh half first so
        # h_proj unblocks while We still streams ----
        wt_bf = const.tile([P, 2 * n_hc, h], BF16)
        wt_r = wt.rearrange("(jc p) o -> p jc o", p=P)
        q = n_hc // 2

        def emit_w(c):
            nc.sync.dma_start(
                wt_bf[:, c * q:(c + 1) * q, :],
                wt_r[:, c * q:(c + 1) * q, :])

        emit_w(2)
        emit_w(3)

        def emit_hproj():
            hb = const.tile([P, n_ot, b_loc], F32, name="hb")
            for ot in range(n_ot):
                ph = psmall.tile([P, b_loc], F32, name="ph", tag="ps")
                for hc in range(n_hc):
                    nc.tensor.matmul(
                        ph[:], wt_bf[:, hc, ot * P:(ot + 1) * P], hT_bf[:, hc, :],
                        start=(hc == 0), stop=(hc == n_hc - 1))
                nc.vector.tensor_tensor(
                    hb[:, ot, :], ph[:],
                    baT[:, ot, None].to_broadcast((P, b_loc)),
                    mybir.AluOpType.add)
            return hb

        # ---- main pipeline pools ----
        inp = ctx.enter_context(tc.tile_pool(name="inp", bufs=inp_bufs))
        bfp = ctx.enter_context(tc.tile_pool(name="bfp", bufs=bfp_bufs))
        if transpose_mode == "dram":
            dram = ctx.enter_context(tc.tile_pool(name="dram", bufs=4, space="DRAM"))
        encT_p = ctx.enter_context(tc.tile_pool(name="encT", bufs=encT_bufs))
        en_p = ctx.enter_context(tc.tile_pool(name="energy", bufs=3))
        pe_p = ctx.enter_context(tc.tile_pool(name="psum_e", bufs=pe_bufs, space="PSUM"))
        pa_p = ctx.enter_context(tc.tile_pool(name="psum_att", bufs=2, space="PSUM"))

        att_rows = const.tile([b_loc, s], F32)

        units = [(b, isb) for b in range(b_loc) for isb in range(n_sb)]

        def phase1(unit):
            b, isb = unit
            sl = slice(isb * sb, (isb + 1) * sb)
            it = inp.tile([P, si_n, h], F32, name="it")
            nc.sync.dma_start(
                it[:], enc[b, sl, :].rearrange("(si p) h -> p si h", p=P))
            bt = bfp.tile([P, si_n, h], BF16, name="bt")
            nc.vector.tensor_copy(out=bt[:], in_=it[:])
            return bt

        def phase2(bt):
            eT = encT_p.tile([P, n_hc, sb], BF16, name="eT")
            for si in range(si_n):
                nc.sync.dma_start_transpose(
                    eT[:, :, si * P:(si + 1) * P], bt[:, si, :])
            return eT

        def phase3(unit, eT, hb):
            b, isb = unit
            sl = slice(isb * sb, (isb + 1) * sb)
            pa_full = pa_p.tile([P, sb], F32, name="pa")
            pa = pa_full[0:1, :]
            pending = None  # v-dot lags one ot-group so tanh is long done
            for ot in range(n_ot):
                for sc in range(n_sc):
                    scl = slice(sc * SC, (sc + 1) * SC)
                    pe = pe_p.tile([P, SC], F32, name="pe")
                    for hc in range(n_hc):
                        nc.tensor.matmul(
                            pe[:], wt_bf[:, n_hc + hc, ot * P:(ot + 1) * P],
                            eT[:, hc, scl],
                            start=(hc == 0), stop=(hc == n_hc - 1))
                    eng = en_p.tile([P, SC], BF16, name="eng")
                    nc.scalar.activation(
                        eng[:], pe[:], AF.Tanh, bias=hb[:, ot, b:b + 1])
                    if pending is not None:
                        pot, peng, pscl = pending
                        nc.tensor.matmul(
                            pa[0:1, pscl], vt_bf[:, pot:pot + 1], peng[:],
                            start=(pot == 0), stop=False,
                            skip_group_check=True)
                    pending = (ot, eng, scl)
            pot, peng, pscl = pending
            nc.tensor.matmul(
                pa[0:1, pscl], vt_bf[:, pot:pot + 1], peng[:],
                start=(pot == 0), stop=True,
                skip_group_check=True)
            att_sb = en_p.tile([1, sb], F32, name="att_sb")
            nc.scalar.activation(att_sb[:], pa[:], AF.Copy)
            nc.gpsimd.dma_start(att_rows[b:b + 1, sl], att_sb[:])

        def keepalive(n):
            for _ in range(n):
                pw = psmall.tile([P, SC], F32, name="pw", tag="ps")
                nc.tensor.matmul(pw[:], wz[:, :P], wz[:], start=True, stop=True)

        # staged startup: u0 alone (smallest xbar-drain set), then u1-2,
        # then steady-state groups; We and h_proj interleave so the PE
        # stream has no hole wider than the HAM window
        bt0 = phase1(units[0])
        eT0 = phase2(bt0)
        emit_w(0)
        emit_w(1)
        hb = emit_hproj()
        phase3(units[0], eT0, hb)
        keepalive(startup_keep)

        rest = units[3:]
        groups = [rest[i:i + group_units]
                  for i in range(0, len(rest), group_units)]

        # software-pipelined: group g's transposes run first, then group
        # g+1's plain copies stream while group g's matmuls execute — the
        # xbar-mode drain pairs (copies <-> transposes) never block the PE
        mid = units[1:3]
        bt12 = [phase1(u) for u in mid]
        eT12 = [phase2(bt) for bt in bt12]
        bts_next = [phase1(u) for u in groups[0]] if groups else []
        for u, eT in zip(mid, eT12):
            phase3(u, eT, hb)
        keepalive(startup_keep)

        for gi, group in enumerate(groups):
            eTs = [phase2(bt) for bt in bts_next]
            if gi + 1 < len(groups):
                bts_next = [phase1(u) for u in groups[gi + 1]]
            for u, eT in zip(group, eTs):
                phase3(u, eT, hb)
            keepalive(keepalive_mm)

        # ---- softmax over s per batch row ----
        mneg = const.tile([b_loc, 1], F32)
        nc.vector.tensor_reduce(
            mneg[:], att_rows[:], mybir.AxisListType.X, mybir.AluOpType.max)
        nc.vector.tensor_scalar_mul(mneg[:], mneg[:], -1.0)
        e_rows = const.tile([b_loc, s], F32)
        ssum = const.tile([b_loc, 1], F32)
        nc.scalar.activation(
            e_rows[:], att_rows[:], AF.Exp, bias=mneg[:], accum_out=ssum[:])
        rinv = const.tile([b_loc, 1], F32)
        nc.vector.reciprocal(rinv[:], ssum[:])
        o_rows = const.tile([b_loc, s], F32)
        nc.vector.tensor_scalar_mul(o_rows[:], e_rows[:], rinv[:])
        nc.sync.dma_start(out[:, :], o_rows[:])

    nc.compile()
    return nc


def make_in_maps(hidden, encoder_outputs, W_attn, b_attn, v, n_cores=8):
    hidden = np.asarray(hidden, dtype=np.float32)
    encoder_outputs = np.asarray(encoder_outputs, dtype=np.float32)
    W_attn = np.asarray(W_attn, dtype=np.float32)
    b_attn = np.asarray(b_attn, dtype=np.float32)
    v = np.asarray(v, dtype=np.float32)

    b = encoder_outputs.shape[0]
    b_loc = b // n_cores
    wt = np.ascontiguousarray(W_attn.T.astype(ml_dtypes.bfloat16))
    v_bf = v.astype(ml_dtypes.bfloat16)
    in_maps = []
    for i in range(n_cores):
        bsl = slice(b_loc * i, b_loc * (i + 1))
        in_maps.append({
            "wt": wt,
            "hiddenT": np.ascontiguousarray(
                hidden[0, bsl].T.astype(ml_dtypes.bfloat16)),
            "b_attn": b_attn,
            "v": v_bf,
            "enc": np.ascontiguousarray(encoder_outputs[bsl]),
        })
    return in_maps


_NC_CACHE = {}


def _get_nc():
    if "nc" not in _NC_CACHE:
        _NC_CACHE["nc"] = build_nc(b_loc=4, s=2048, h=1024, n_cores=8)
    return _NC_CACHE["nc"]


def kernel(hidden, encoder_outputs, W_attn, b_attn, v):
    from concourse.bass_utils import run_bass_kernel_spmd

    nc = _get_nc()
    in_maps = make_in_maps(hidden, encoder_outputs, W_attn, b_attn, v,
                           n_cores=8)
    res = run_bass_kernel_spmd(nc, in_maps, core_ids=list(range(8)))
    out = np.concatenate([np.asarray(res.results[i]["out"])
                          for i in range(8)], axis=0)
    return out.astype(np.float32)

